# revision 13
# baseline (speedup 1.0000x reference)
"""Trainium2 Bass kernel for nn_MultiHeadHCGAttention.

Math notes (exact restructuring of the reference):
  The key_padding_mask replaces the ENTIRE key feature row with -1e9 BEFORE
  the K projection (v is NOT masked). Hence every masked key position s in
  batch b has the SAME projected K row:
      Kmask[n] = -1e9 * sum_h Wk[n,h,:] + bk[n]   (data independent)
  All masked keys share one score z = Q.Kmask/sqrt(dk) with |z| ~ 1e9.
  In fp32 softmax the output per (query q, head n) is therefore either
    - mean of V over the masked key positions  if z > max unmasked score
      (uniform softmax over the identical-score masked keys)
    - standard softmax over unmasked keys      otherwise (masked weights
      underflow to exactly 0 in fp32)
  The boundary band has probability ~1e-7 per query -> decided by sign(z),
  computed exactly on the host in fp64 (z = q @ (Wq@Kmask) + bq.Kmask).

  Device computes bf16 attention over the gathered unmasked keys only
  (normal O(1) magnitudes); rows whose head chose the mask branch get
  denom += 1e30 on device (output underflows to 0) and the contribution
  ubar[b,n] = (mean_masked V[b,n]) @ Wo_n is added on the host in fp64.
  bv is folded into bo on the host (softmax weights sum to 1 so
  out = PV/d + bv exactly before the output projection).

Sharding: 8 cores = (batch b in 0..3) x (query half). The two cores of a
batch each project only HALF the unmasked keys (K and V projections) and
exchange the projected halves via a pairwise AllGather, removing the
duplicated K/V projection work. The softmax denominator is a cheap DVE
bf16 chain over the exp tiles + one 512-cycle matmul (not a per-k-tile
ones-matmul), deferred one chunk so the PE never waits on the DVE chain.
Attention runs qc-major so each query-chunk's output projection overlaps
the next chunk's (Scalar-engine-bound) exp work.
"""

import math
import sys

if "/opt/trn_rl_repo" not in sys.path:
    sys.path.insert(0, "/opt/trn_rl_repo")

import ml_dtypes
import numpy as np

import concourse.bacc as bacc
import concourse.tile as tile
from concourse import mybir
from concourse.bass_utils import run_bass_kernel_spmd

S, B, H = 2048, 4, 1024
NH, DK = 8, 128
NHDK = NH * DK
NEG = -1.0e9
NCORES = 8
HT = H // 128  # 8 H-tiles
GROUPS = [[0, 1], [2, 3], [4, 5], [6, 7]]

bf16 = mybir.dt.bfloat16
f32 = mybir.dt.float32
npbf16 = ml_dtypes.bfloat16

_PROG_CACHE: dict = {}


def _split(UP):
    NKT = (UP + 127) // 128
    BKT = NKT // 2  # full k-tiles in half 0
    BOUND = BKT * 128
    HKT = NKT - BKT  # k-tiles in half 1 (>= BKT)
    UPH = max(BOUND, UP - BOUND)  # per-half padded width (uniform program)
    return NKT, BKT, BOUND, HKT, UPH


def build_program(Sq: int, UP: int):
    """Per-core SPMD program. Sq = queries per core; UP = unmasked-key count
    (max over batches). Each core projects only its k-half (width UPH,
    host-sliced input) and receives the other half via pairwise AllGather."""
    NKT, BKT, BOUND, HKT, UPH = _split(UP)
    ktiles = [(o, min(128, UP - o)) for o in range(0, UP, 128)]  # attention
    htiles = [(o, min(128, UPH - o)) for o in range(0, UPH, 128)]  # own half
    QC = Sq // 512
    # free-dim chunks for the K projection over the own half
    hchunks = []
    o = 0
    while o < UPH:
        w = min(512, UPH - o)
        hchunks.append((o, w))
        o += w

    nc = bacc.Bacc("TRN2", target_bir_lowering=False, debug=False, num_devices=8)

    d_qT = nc.dram_tensor("qT", [H, Sq], bf16, kind="ExternalInput")
    d_kuT = nc.dram_tensor("kuT", [H, UPH], bf16, kind="ExternalInput")
    d_vuT = nc.dram_tensor("vuT", [H, UPH], bf16, kind="ExternalInput")
    d_wq = nc.dram_tensor("wq", [H, NHDK], bf16, kind="ExternalInput")
    d_wk = nc.dram_tensor("wk", [H, NHDK], bf16, kind="ExternalInput")
    d_wv = nc.dram_tensor("wv", [H, NHDK], bf16, kind="ExternalInput")
    d_wo = nc.dram_tensor("wo", [NHDK, H], bf16, kind="ExternalInput")
    d_bq = nc.dram_tensor("bq", [DK, NH], f32, kind="ExternalInput")
    d_bk = nc.dram_tensor("bk", [DK, NH], f32, kind="ExternalInput")
    d_bo = nc.dram_tensor("bo", [128, HT], f32, kind="ExternalInput")
    d_padb = nc.dram_tensor("padb", [128, NKT], f32, kind="ExternalInput")
    d_chb = nc.dram_tensor("chb", [1, NH * Sq], f32, kind="ExternalInput")
    d_yT = nc.dram_tensor("yT", [H, Sq], f32, kind="ExternalOutput")

    SCALE = 1.0 / math.sqrt(DK)

    with tile.TileContext(nc) as tc:
        with (
            tc.tile_pool(name="const", bufs=1) as const,
            tc.tile_pool(name="exp", bufs=3) as expp,
            tc.tile_pool(name="es", bufs=2) as esp,
            tc.tile_pool(name="sc", bufs=3) as scp,
            tc.tile_pool(name="bc", bufs=2) as bcp,
            tc.tile_pool(name="yt", bufs=3) as ytp,
            tc.tile_pool(name="dram", bufs=6, space="DRAM") as dram,
            tc.tile_pool(name="ps_proj", bufs=3, space="PSUM") as ps_proj,
            tc.tile_pool(name="ps_pv", bufs=3, space="PSUM") as ps_pv,
            tc.tile_pool(name="ps_d", bufs=2, space="PSUM") as ps_d,
        ):
            # ---- SBUF residents ----
            qT = const.tile([128, HT, Sq], bf16)
            kuT = const.tile([128, HT, UPH], bf16)
            vuT = const.tile([128, HT, UPH], bf16)
            wq = const.tile([128, HT, NHDK], bf16)
            wk = const.tile([128, HT, NHDK], bf16)
            wv = const.tile([128, HT, NHDK], bf16)
            wo = const.tile([128, NH, H], bf16)
            bq = const.tile([128, NH], f32)
            bk = const.tile([128, NH], f32)
            bo = const.tile([128, HT], f32)
            padb = const.tile([128, NKT], f32)
            ones_mat = const.tile([128, 128], bf16)
            nc.vector.memset(ones_mat[:], 1.0)
            # full projected K/V (own half written in place, then the
            # AllGather round-trip overwrites with both halves)
            ksb = const.tile([128, NH, UP], bf16)
            vg = const.tile([128, 2, NKT, 512], bf16)
            qsb = const.tile([128, NH, Sq], bf16)
            out_all = const.tile([128, NH, Sq], bf16)

            r_qT = d_qT[:].rearrange("(t p) s -> p t s", p=128)
            r_kuT = d_kuT[:].rearrange("(t p) u -> p t u", p=128)
            r_vuT = d_vuT[:].rearrange("(t p) u -> p t u", p=128)
            r_wq = d_wq[:].rearrange("(t p) d -> p t d", p=128)
            r_wk = d_wk[:].rearrange("(t p) d -> p t d", p=128)
            r_wv = d_wv[:].rearrange("(t p) d -> p t d", p=128)
            r_wo = d_wo[:].rearrange("(n p) h -> p n h", p=128)

            # Single SP HWDGE ring, strict consumption order (FIFO split
            # across all 16 SDMA engines): K-projection inputs first, then
            # V, then Q, output weights last.
            for ht in range(HT):
                nc.sync.dma_start(kuT[:, ht, :], r_kuT[:, ht, :])
                nc.sync.dma_start(wk[:, ht, :], r_wk[:, ht, :])
                if ht == 0:
                    nc.sync.dma_start(bq[:], d_bq[:])
                    nc.sync.dma_start(bk[:], d_bk[:])
                    nc.sync.dma_start(bo[:], d_bo[:])
                    nc.sync.dma_start(padb[:], d_padb[:])
            for ht in range(HT):
                nc.sync.dma_start(vuT[:, ht, :], r_vuT[:, ht, :])
                nc.sync.dma_start(wv[:, ht, :], r_wv[:, ht, :])
            for ht in range(HT):
                nc.sync.dma_start(qT[:, ht, :], r_qT[:, ht, :])
                nc.sync.dma_start(wq[:, ht, :], r_wq[:, ht, :])
            for n in range(NH):
                nc.sync.dma_start(wo[:, n, :], r_wo[:, n, :])

            # ---- own-half K projection (all heads) ----
            for n in range(NH):
                for o, w in hchunks:
                    pk = ps_proj.tile([128, 512], f32, tag="proj")
                    for ht in range(HT):
                        nc.tensor.matmul(
                            pk[:, :w],
                            wk[:, ht, n * 128 : (n + 1) * 128],
                            kuT[:, ht, o : o + w],
                            start=(ht == 0),
                            stop=(ht == HT - 1),
                        )
                    nc.vector.tensor_scalar_add(
                        ksb[:, n, o : o + w], pk[:, :w], bk[:, n : n + 1]
                    )
            # stage + pairwise AllGather of the projected K half
            stgK = dram.tile([128, NH, UPH], bf16, tag="stgK")
            gthK = dram.tile([2, 128, NH, UPH], bf16, tag="gthK")
            nc.gpsimd.dma_start(stgK[:], ksb[:, :, 0:UPH])
            nc.gpsimd.collective_compute(
                "AllGather",
                mybir.AluOpType.bypass,
                replica_groups=GROUPS,
                ins=[stgK[:]],
                outs=[gthK[:]],
            )

            # ---- own-half V projection (both head groups) ----
            gthV = {}
            for g in range(2):
                for kt, (o, klen) in enumerate(htiles):
                    pv = ps_proj.tile([128, 512], f32, tag="proj")
                    for ht in range(HT):
                        nc.tensor.matmul(
                            pv[:klen],
                            vuT[:, ht, o : o + klen],
                            wv[:, ht, g * 512 : (g + 1) * 512],
                            start=(ht == 0),
                            stop=(ht == HT - 1),
                        )
                    nc.vector.tensor_copy(vg[:klen, g, kt, :], pv[:klen])
                stgV = dram.tile([128, HKT, 512], bf16, tag=f"stgV{g}")
                gthVg = dram.tile([2, 128, HKT, 512], bf16, tag=f"gthV{g}")
                gthV[g] = gthVg
                nc.gpsimd.dma_start(stgV[:], vg[:, g, 0:HKT, :])
                nc.gpsimd.collective_compute(
                    "AllGather",
                    mybir.AluOpType.bypass,
                    replica_groups=GROUPS,
                    ins=[stgV[:]],
                    outs=[gthV[g][:]],
                )

            # ---- Q projection (overlaps the collectives) ----
            for n in range(NH):
                for qc in range(QC):
                    pq = ps_proj.tile([128, 512], f32, tag="proj")
                    for ht in range(HT):
                        nc.tensor.matmul(
                            pq[:],
                            wq[:, ht, n * 128 : (n + 1) * 128],
                            qT[:, ht, qc * 512 : (qc + 1) * 512],
                            start=(ht == 0),
                            stop=(ht == HT - 1),
                        )
                    nc.vector.tensor_scalar_add(
                        qsb[:, n, qc * 512 : (qc + 1) * 512], pq[:], bq[:, n : n + 1]
                    )

            # ---- read back both halves from the gathered buffers ----
            nc.gpsimd.dma_start(ksb[:, :, 0:BOUND], gthK[0, :, :, 0:BOUND])
            nc.gpsimd.dma_start(ksb[:, :, BOUND:UP], gthK[1, :, :, 0 : UP - BOUND])
            for g in range(2):
                nc.gpsimd.dma_start(vg[:, g, 0:BKT, :], gthV[g][0, :, 0:BKT, :])
                nc.gpsimd.dma_start(
                    vg[:, g, BKT:NKT, :], gthV[g][1, :, 0 : NKT - BKT, :]
                )

            # ---- attention ----
            def attn_chunk(n, qc):
                """Scores + exp + PV for one (head, 512-query) chunk. The
                softmax denominator accumulates as a DVE bf16 chain over the
                exp tiles; its single matmul is deferred one chunk so the PE
                never waits on the chain."""
                qsl = slice(qc * 512, (qc + 1) * 512)
                chb = bcp.tile([128, 512], f32, tag="chb")
                nc.gpsimd.dma_start(
                    chb[:],
                    d_chb[
                        0:1, n * Sq + qc * 512 : n * Sq + (qc + 1) * 512
                    ].to_broadcast([128, 512]),
                )
                ppv = ps_pv.tile([128, 512], f32)
                esum = esp.tile([128, 512], bf16)
                e0 = None
                k0 = 0
                for kt, (ko, klen) in enumerate(ktiles):
                    ps = ps_proj.tile([128, 512], f32, tag="proj")
                    nc.tensor.matmul(
                        ps[:klen],
                        ksb[:, n, ko : ko + klen],
                        qsb[:, n, qsl],
                        start=True,
                        stop=True,
                    )
                    e = expp.tile([128, 512], bf16)
                    nc.scalar.activation(
                        out=e[:klen],
                        in_=ps[:klen],
                        func=mybir.ActivationFunctionType.Exp,
                        bias=padb[:klen, kt : kt + 1],
                        scale=SCALE,
                    )
                    nc.tensor.matmul(
                        ppv[:],
                        vg[:klen, n // 4, kt, n % 4 * 128 : (n % 4 + 1) * 128],
                        e[:klen],
                        start=(kt == 0),
                        stop=(kt == NKT - 1),
                    )
                    if kt == 0:
                        e0, k0 = e, klen
                    elif kt == 1:
                        nc.vector.tensor_add(esum[:klen], e0[:klen], e[:klen])
                        if klen < k0:
                            nc.vector.tensor_copy(esum[klen:k0], e0[klen:k0])
                    else:
                        nc.vector.tensor_add(esum[:klen], esum[:klen], e[:klen])
                if NKT == 1:
                    nc.vector.tensor_copy(esum[:k0], e0[:k0])
                return (n, qsl, ppv, esum, chb)

            KMAX = min(128, UP)

            def attn_finish(st):
                n, qsl, ppv, esum, chb = st
                pd = ps_d.tile([128, 512], f32)
                nc.tensor.matmul(
                    pd[:], ones_mat[:KMAX], esum[:KMAX], start=True, stop=True
                )
                # mask-branch rows get denom += 1e30: output underflows to 0
                pda = scp.tile([128, 512], f32, tag="pda")
                nc.vector.tensor_add(pda[:], pd[:], chb[:])
                rec = scp.tile([128, 512], f32, tag="rec")
                nc.vector.reciprocal_approx_fast(rec[:], pda[:])
                nc.vector.tensor_mul(out_all[:, n, qsl], ppv[:], rec[:])

            r_yT = d_yT[:].rearrange("(t p) s -> t p s", p=128)

            def outproj_piece(qc, ht):
                py = ps_proj.tile([128, 512], f32, tag="proj")
                for n in range(NH):
                    nc.tensor.matmul(
                        py[:],
                        wo[:, n, ht * 128 : (ht + 1) * 128],
                        out_all[:, n, qc * 512 : (qc + 1) * 512],
                        start=(n == 0),
                        stop=(n == NH - 1),
                    )
                yt = ytp.tile([128, 512], f32)
                nc.vector.tensor_scalar_add(yt[:], py[:], bo[:, ht : ht + 1])
                nc.sync.dma_start(
                    r_yT[ht, :, qc * 512 : (qc + 1) * 512], yt[:]
                )

            # qc-major: qc0 heads 0..7, then qc1 heads with qc0's output
            # projection pieces interleaved (keeps the Scalar engine fed).
            pending = None
            for qc in range(QC):
                for n in range(NH):
                    st = attn_chunk(n, qc)
                    if pending is not None:
                        attn_finish(pending)
                    pending = st
                    if qc > 0 and n >= 1:
                        outproj_piece(qc - 1, n - 1)
            attn_finish(pending)
            for qc_prev in range(QC - 1):
                outproj_piece(qc_prev, HT - 1)
            for ht in range(HT):
                outproj_piece(QC - 1, ht)

    nc.compile()
    return nc


def _prepare(query, key, value, key_padding_mask, Wq, bq, Wk, bk, Wv, bv, Wo, bo):
    """Host-side prep: mask constants (fp64), gather/transpose, per-core maps."""
    mask = np.asarray(key_padding_mask)
    q64 = np.asarray(query, np.float64)
    Wq64 = np.asarray(Wq, np.float64)
    Wk64 = np.asarray(Wk, np.float64)
    Wv64 = np.asarray(Wv, np.float64)
    Wo64 = np.asarray(Wo, np.float64)

    # shared projected row of all masked keys, per head
    kmask = NEG * Wk64.sum(axis=1) + np.asarray(bk, np.float64)  # [NH, DK]

    # z sign per (s, b, n):  z = q . (Wq[n] @ kmask[n]) + bq[n].kmask[n]
    wz = np.einsum("nhd,nd->hn", Wq64, kmask)  # [H, NH]
    cz = np.einsum("nd,nd->n", np.asarray(bq, np.float64), kmask)  # [NH]
    z = q64.reshape(S * B, H) @ wz + cz  # [S*B, NH]
    choose = (z > 0).reshape(S, B, NH)

    # mask-branch output: mean of (unmasked-data) V over masked key positions
    v64 = np.asarray(value, np.float64)  # [S, B, H]
    vbar_feat = np.stack(
        [
            v64[mask[b], b, :].mean(axis=0)
            if mask[b].any()
            else np.zeros(H)
            for b in range(B)
        ]
    )  # [B, H]
    for b in range(B):
        if not mask[b].any():
            choose[:, b, :] = False  # no masked keys -> no mask branch
        elif mask[b].all():
            # all keys masked: identical scores -> uniform softmax -> Vbar
            choose[:, b, :] = True
    # bv is folded into bo on the device, so the host correction uses vbar
    # WITHOUT bv (the device adds bv@Wo to every row via the output bias).
    vbar = np.einsum("bh,nhd->bnd", vbar_feat, Wv64)  # [B, NH, DK]
    ubar = np.einsum(
        "bnd,ndh->bnh", vbar, Wo64.reshape(NH, DK, H)
    )  # [B, NH, H]

    # correction added on host for mask-branch rows
    ycorr = np.einsum("sbn,bnh->sbh", choose.astype(np.float64), ubar)

    # gather unmasked keys per batch
    idx = [np.nonzero(~mask[b])[0] for b in range(B)]
    umax = max(max(len(i) for i in idx), 1)
    UP = umax
    NKT, BKT, BOUND, HKT, UPH = _split(UP)

    Wq_d = np.ascontiguousarray(
        np.asarray(Wq).transpose(1, 0, 2).reshape(H, NHDK)
    ).astype(npbf16)
    Wk_d = np.ascontiguousarray(
        np.asarray(Wk).transpose(1, 0, 2).reshape(H, NHDK)
    ).astype(npbf16)
    Wv_d = np.ascontiguousarray(
        np.asarray(Wv).transpose(1, 0, 2).reshape(H, NHDK)
    ).astype(npbf16)
    Wo_d = np.asarray(Wo, np.float32).astype(npbf16)
    bq_d = np.ascontiguousarray(np.asarray(bq, np.float32).T)  # [DK, NH]
    bk_d = np.ascontiguousarray(np.asarray(bk, np.float32).T)
    # fold bv into the output bias: y += bv_flat @ Wo
    bo_eff = np.asarray(bo, np.float64) + np.asarray(bv, np.float64).reshape(
        NHDK
    ) @ Wo64
    bo_d = np.ascontiguousarray(
        bo_eff.astype(np.float32).reshape(HT, 128).T
    )  # [128, HT]

    Sq = S // 2
    in_maps = []
    for core in range(NCORES):
        b, half = divmod(core, 2)
        qo = half * Sq
        ii = idx[b]
        u = len(ii)
        # this core projects k-half `half`: [0:BOUND] or [BOUND:UP]
        lo = 0 if half == 0 else min(BOUND, u)
        hi = min(BOUND, u) if half == 0 else u
        sel = ii[lo:hi]
        w = len(sel)
        kuT = np.zeros((H, UPH), npbf16)
        vuT = np.zeros((H, UPH), npbf16)
        if w > 0:
            kuT[:, :w] = np.asarray(key[sel, b, :], np.float32).T.astype(npbf16)
            vuT[:, :w] = np.asarray(value[sel, b, :], np.float32).T.astype(npbf16)
        qT = np.ascontiguousarray(
            np.asarray(query[qo : qo + Sq, b, :], np.float32).T
        ).astype(npbf16)
        padb = np.zeros((128, NKT), np.float32)
        flat = np.arange(NKT * 128).reshape(NKT, 128).T  # [128, NKT] key index
        padb[flat >= max(u, 1)] = -30000.0  # keep >=1 live key (denom > 0)
        chb = np.ascontiguousarray(
            choose[qo : qo + Sq, b, :].T.astype(np.float32) * 1.0e30
        ).reshape(1, NH * Sq)
        in_maps.append(
            {
                "qT": qT,
                "kuT": kuT,
                "vuT": vuT,
                "wq": Wq_d,
                "wk": Wk_d,
                "wv": Wv_d,
                "wo": Wo_d,
                "bq": bq_d,
                "bk": bk_d,
                "bo": bo_d,
                "padb": padb,
                "chb": chb,
            }
        )
    return in_maps, ycorr, Sq, UP


def run(inputs: dict, trace: bool = False):
    in_maps, ycorr, Sq, UP = _prepare(**inputs)
    key_ = (Sq, UP)
    if key_ not in _PROG_CACHE:
        _PROG_CACHE[key_] = build_program(Sq, UP)
    nc = _PROG_CACHE[key_]
    res = run_bass_kernel_spmd(nc, in_maps, list(range(NCORES)), trace=trace)
    y = np.empty((S, B, H), np.float32)
    for core in range(NCORES):
        b, half = divmod(core, 2)
        qo = half * Sq
        y[qo : qo + Sq, b, :] = res.results[core]["yT"].T
    y += ycorr.astype(np.float32)
    return y, res


def kernel(**inputs) -> np.ndarray:
    y, _ = run(inputs, trace=False)
    return y


# revision 17
# speedup vs baseline: 1.0367x; 1.0367x over previous
"""Trainium2 Bass kernel for nn_MultiHeadHCGAttention.

Math notes (exact restructuring of the reference):
  The key_padding_mask replaces the ENTIRE key feature row with -1e9 BEFORE
  the K projection (v is NOT masked). Hence every masked key position s in
  batch b has the SAME projected K row:
      Kmask[n] = -1e9 * sum_h Wk[n,h,:] + bk[n]   (data independent)
  All masked keys share one score z = Q.Kmask/sqrt(dk) with |z| ~ 1e9.
  In fp32 softmax the output per (query q, head n) is therefore either
    - mean of V over the masked key positions  if z > max unmasked score
      (uniform softmax over the identical-score masked keys)
    - standard softmax over unmasked keys      otherwise (masked weights
      underflow to exactly 0 in fp32)
  The boundary band has probability ~1e-7 per query -> decided by sign(z),
  computed exactly on the host in fp64 (z = q @ (Wq@Kmask) + bq.Kmask).

  Device computes bf16 attention over the gathered unmasked keys only
  (normal O(1) magnitudes); rows whose head chose the mask branch get
  denom += 1e30 on device (output underflows to 0) and the contribution
  ubar[b,n] = (mean_masked V[b,n]) @ Wo_n is added on the host in fp64.
  bv is folded into bo on the host (softmax weights sum to 1 so
  out = PV/d + bv exactly before the output projection).

Sharding: 8 cores = (batch b in 0..3) x (query half). No collectives (the
pairwise AllGather fabric here moves ~38GB/s -- too slow to pay for
de-duplicating the K/V projections).

Schedule: projections and attention are interleaved so the Scalar engine's
exp stream (the attention-phase bottleneck) overlaps projection matmuls,
and attention runs qc-major with the first query-chunk's output projection
emitted piecewise between second-chunk heads. The softmax denominator is a
DVE bf16 chain over the exp tiles + one 512-cycle matmul per chunk
(deferred one chunk so the PE never waits on the chain).
"""

import math
import sys

if "/opt/trn_rl_repo" not in sys.path:
    sys.path.insert(0, "/opt/trn_rl_repo")

import ml_dtypes
import numpy as np

import concourse.bacc as bacc
import concourse.tile as tile
from concourse import mybir
from concourse.bass_utils import run_bass_kernel_spmd

S, B, H = 2048, 4, 1024
NH, DK = 8, 128
NHDK = NH * DK
NEG = -1.0e9
NCORES = 8
HT = H // 128  # 8 H-tiles

bf16 = mybir.dt.bfloat16
f32 = mybir.dt.float32
npbf16 = ml_dtypes.bfloat16

_PROG_CACHE: dict = {}


def build_program(Sq: int, UP: int):
    """Emit the per-core SPMD program. Sq = queries per core, UP =
    unmasked-key count (max over batches)."""
    NKT = (UP + 127) // 128
    ktiles = [(o, min(128, UP - o)) for o in range(0, UP, 128)]
    QC = Sq // 512  # 512-wide query chunks
    # key free-dim chunks for the K projection
    kchunks = []
    o = 0
    while o < UP:
        w = min(512, UP - o)
        kchunks.append((o, w))
        o += w

    nc = bacc.Bacc("TRN2", target_bir_lowering=False, debug=False)

    d_qT = nc.dram_tensor("qT", [H, Sq], bf16, kind="ExternalInput")
    d_kuT = nc.dram_tensor("kuT", [H, UP], bf16, kind="ExternalInput")
    d_vuT = nc.dram_tensor("vuT", [H, UP], bf16, kind="ExternalInput")
    d_wq = nc.dram_tensor("wq", [H, NHDK], bf16, kind="ExternalInput")
    d_wk = nc.dram_tensor("wk", [H, NHDK], bf16, kind="ExternalInput")
    d_wv = nc.dram_tensor("wv", [H, NHDK], bf16, kind="ExternalInput")
    d_wo = nc.dram_tensor("wo", [NHDK, H], bf16, kind="ExternalInput")
    d_bq = nc.dram_tensor("bq", [DK, NH], f32, kind="ExternalInput")
    d_bk = nc.dram_tensor("bk", [DK, NH], f32, kind="ExternalInput")
    d_bo = nc.dram_tensor("bo", [128, HT], f32, kind="ExternalInput")
    d_padb = nc.dram_tensor("padb", [128, NKT], f32, kind="ExternalInput")
    d_chb = nc.dram_tensor("chb", [1, NH * Sq], f32, kind="ExternalInput")
    d_yT = nc.dram_tensor("yT", [H, Sq], f32, kind="ExternalOutput")

    SCALE = 1.0 / math.sqrt(DK)

    with tile.TileContext(nc) as tc:
        with (
            tc.tile_pool(name="const", bufs=1) as const,
            tc.tile_pool(name="exp", bufs=3) as expp,
            tc.tile_pool(name="es", bufs=2) as esp,
            tc.tile_pool(name="sc", bufs=3) as scp,
            tc.tile_pool(name="bc", bufs=2) as bcp,
            tc.tile_pool(name="yt", bufs=3) as ytp,
            tc.tile_pool(name="ps_proj", bufs=3, space="PSUM") as ps_proj,
            tc.tile_pool(name="ps_pv", bufs=3, space="PSUM") as ps_pv,
            tc.tile_pool(name="ps_d", bufs=2, space="PSUM") as ps_d,
        ):
            qT = const.tile([128, HT, Sq], bf16)
            kuT = const.tile([128, HT, UP], bf16)
            vuT = const.tile([128, HT, UP], bf16)
            wq = const.tile([128, HT, NHDK], bf16)
            wk = const.tile([128, HT, NHDK], bf16)
            wv = const.tile([128, HT, NHDK], bf16)
            wo = const.tile([128, NH, H], bf16)
            bq = const.tile([128, NH], f32)
            bk = const.tile([128, NH], f32)
            bo = const.tile([128, HT], f32)
            padb = const.tile([128, NKT], f32)
            ones_mat = const.tile([128, 128], bf16)
            nc.vector.memset(ones_mat[:], 1.0)
            ksb = const.tile([128, NH, UP], bf16)
            vg = const.tile([128, 2, NKT, 512], bf16)
            qsb = const.tile([128, NH, Sq], bf16)
            out_all = const.tile([128, NH, Sq], bf16)

            r_qT = d_qT[:].rearrange("(t p) s -> p t s", p=128)
            r_kuT = d_kuT[:].rearrange("(t p) u -> p t u", p=128)
            r_vuT = d_vuT[:].rearrange("(t p) u -> p t u", p=128)
            r_wq = d_wq[:].rearrange("(t p) d -> p t d", p=128)
            r_wk = d_wk[:].rearrange("(t p) d -> p t d", p=128)
            r_wv = d_wv[:].rearrange("(t p) d -> p t d", p=128)
            r_wo = d_wo[:].rearrange("(n p) h -> p n h", p=128)

            # Single SP HWDGE ring, strict consumption order (FIFO, split
            # across all 16 SDMA engines). kuT/wk interleaved per H-tile so
            # the first kproj chain starts ASAP; later groups batched into
            # half-tensor transfers to cut trigger overhead on the queue.
            for ht in range(HT):
                nc.sync.dma_start(kuT[:, ht, :], r_kuT[:, ht, :])
                nc.sync.dma_start(wk[:, ht, :], r_wk[:, ht, :])
            nc.sync.dma_start(bq[:], d_bq[:])
            nc.sync.dma_start(bk[:], d_bk[:])
            nc.sync.dma_start(bo[:], d_bo[:])
            nc.sync.dma_start(padb[:], d_padb[:])
            for h0 in (0, 4):
                nc.sync.dma_start(vuT[:, h0 : h0 + 4, :], r_vuT[:, h0 : h0 + 4, :])
                nc.sync.dma_start(wv[:, h0 : h0 + 4, :], r_wv[:, h0 : h0 + 4, :])
            for h0 in (0, 4):
                nc.sync.dma_start(qT[:, h0 : h0 + 4, :], r_qT[:, h0 : h0 + 4, :])
                nc.sync.dma_start(wq[:, h0 : h0 + 4, :], r_wq[:, h0 : h0 + 4, :])
            for n0 in (0, 4):
                nc.sync.dma_start(wo[:, n0 : n0 + 4, :], r_wo[:, n0 : n0 + 4, :])

            def kproj(n):
                for o, w in kchunks:
                    pk = ps_proj.tile([128, 512], f32, tag="proj")
                    for ht in range(HT):
                        nc.tensor.matmul(
                            pk[:, :w],
                            wk[:, ht, n * 128 : (n + 1) * 128],
                            kuT[:, ht, o : o + w],
                            start=(ht == 0),
                            stop=(ht == HT - 1),
                        )
                    nc.vector.tensor_scalar_add(
                        ksb[:, n, o : o + w], pk[:, :w], bk[:, n : n + 1]
                    )

            def vproj(g):
                for kt, (o, klen) in enumerate(ktiles):
                    pv = ps_proj.tile([128, 512], f32, tag="proj")
                    for ht in range(HT):
                        nc.tensor.matmul(
                            pv[:klen],
                            vuT[:, ht, o : o + klen],
                            wv[:, ht, g * 512 : (g + 1) * 512],
                            start=(ht == 0),
                            stop=(ht == HT - 1),
                        )
                    nc.vector.tensor_copy(vg[:klen, g, kt, :], pv[:klen])

            def qproj(n):
                for qc in range(QC):
                    pq = ps_proj.tile([128, 512], f32, tag="proj")
                    for ht in range(HT):
                        nc.tensor.matmul(
                            pq[:],
                            wq[:, ht, n * 128 : (n + 1) * 128],
                            qT[:, ht, qc * 512 : (qc + 1) * 512],
                            start=(ht == 0),
                            stop=(ht == HT - 1),
                        )
                    nc.vector.tensor_scalar_add(
                        qsb[:, n, qc * 512 : (qc + 1) * 512], pq[:], bq[:, n : n + 1]
                    )

            def attn_chunk(n, qc):
                """Scores + exp + PV for one (head, 512-query) chunk. The
                softmax denominator accumulates as a DVE bf16 chain over the
                exp tiles; its single matmul is deferred one chunk so the PE
                never waits on the chain."""
                qsl = slice(qc * 512, (qc + 1) * 512)
                chb = bcp.tile([128, 512], f32, tag="chb")
                nc.gpsimd.dma_start(
                    chb[:],
                    d_chb[
                        0:1, n * Sq + qc * 512 : n * Sq + (qc + 1) * 512
                    ].to_broadcast([128, 512]),
                )
                ppv = ps_pv.tile([128, 512], f32)
                esum = esp.tile([128, 512], bf16)
                e0 = None
                k0 = 0
                for kt, (ko, klen) in enumerate(ktiles):
                    ps = ps_proj.tile([128, 512], f32, tag="proj")
                    nc.tensor.matmul(
                        ps[:klen],
                        ksb[:, n, ko : ko + klen],
                        qsb[:, n, qsl],
                        start=True,
                        stop=True,
                    )
                    e = expp.tile([128, 512], bf16)
                    nc.scalar.activation(
                        out=e[:klen],
                        in_=ps[:klen],
                        func=mybir.ActivationFunctionType.Exp,
                        bias=padb[:klen, kt : kt + 1],
                        scale=SCALE,
                    )
                    nc.tensor.matmul(
                        ppv[:],
                        vg[:klen, n // 4, kt, n % 4 * 128 : (n % 4 + 1) * 128],
                        e[:klen],
                        start=(kt == 0),
                        stop=(kt == NKT - 1),
                    )
                    if kt == 0:
                        e0, k0 = e, klen
                    elif kt == 1:
                        nc.vector.tensor_add(esum[:klen], e0[:klen], e[:klen])
                        if klen < k0:
                            nc.vector.tensor_copy(esum[klen:k0], e0[klen:k0])
                    else:
                        nc.vector.tensor_add(esum[:klen], esum[:klen], e[:klen])
                if NKT == 1:
                    nc.vector.tensor_copy(esum[:k0], e0[:k0])
                return (n, qsl, ppv, esum, chb)

            KMAX = min(128, UP)

            def attn_finish(st):
                n, qsl, ppv, esum, chb = st
                pd = ps_d.tile([128, 512], f32)
                nc.tensor.matmul(
                    pd[:], ones_mat[:KMAX], esum[:KMAX], start=True, stop=True
                )
                # mask-branch rows get denom += 1e30: output underflows to 0
                pda = scp.tile([128, 512], f32, tag="pda")
                nc.vector.tensor_add(pda[:], pd[:], chb[:])
                rec = scp.tile([128, 512], f32, tag="rec")
                nc.vector.reciprocal_approx_fast(rec[:], pda[:])
                nc.vector.tensor_mul(out_all[:, n, qsl], ppv[:], rec[:])

            r_yT = d_yT[:].rearrange("(t p) s -> t p s", p=128)

            def outproj_piece(qc, ht):
                py = ps_proj.tile([128, 512], f32, tag="proj")
                for n in range(NH):
                    nc.tensor.matmul(
                        py[:],
                        wo[:, n, ht * 128 : (ht + 1) * 128],
                        out_all[:, n, qc * 512 : (qc + 1) * 512],
                        start=(n == 0),
                        stop=(n == NH - 1),
                    )
                yt = ytp.tile([128, 512], f32)
                nc.vector.tensor_scalar_add(yt[:], py[:], bo[:, ht : ht + 1])
                nc.sync.dma_start(
                    r_yT[ht, :, qc * 512 : (qc + 1) * 512], yt[:]
                )

            # ---- interleaved schedule ----
            # batch 1 of projections, then attention chunks (qc0, heads 0-3)
            # interleaved with batch 2, then (qc0, heads 4-7), then qc1 with
            # qc0's output projection pieces spliced between heads.
            pending = None

            def emit_chunk(n, qc):
                nonlocal pending
                st = attn_chunk(n, qc)
                if pending is not None:
                    attn_finish(pending)
                pending = st

            for n in range(4):
                kproj(n)
            vproj(0)
            for n in range(4):
                qproj(n)
            for j, n in enumerate(range(4)):
                kproj(4 + j)
                emit_chunk(n, 0)
            vproj(1)
            for j, n in enumerate(range(4, NH)):
                qproj(4 + j)
                emit_chunk(n, 0)
            for n in range(NH):
                emit_chunk(n, 1)
                if n >= 1:
                    outproj_piece(0, n - 1)
            attn_finish(pending)
            outproj_piece(0, HT - 1)
            for ht in range(HT):
                outproj_piece(1, ht)

    nc.compile()
    return nc


def _prepare(query, key, value, key_padding_mask, Wq, bq, Wk, bk, Wv, bv, Wo, bo):
    """Host-side prep: mask constants (fp64), gather/transpose, per-core maps."""
    mask = np.asarray(key_padding_mask)
    q64 = np.asarray(query, np.float64)
    Wq64 = np.asarray(Wq, np.float64)
    Wk64 = np.asarray(Wk, np.float64)
    Wv64 = np.asarray(Wv, np.float64)
    Wo64 = np.asarray(Wo, np.float64)

    # shared projected row of all masked keys, per head
    kmask = NEG * Wk64.sum(axis=1) + np.asarray(bk, np.float64)  # [NH, DK]

    # z sign per (s, b, n):  z = q . (Wq[n] @ kmask[n]) + bq[n].kmask[n]
    wz = np.einsum("nhd,nd->hn", Wq64, kmask)  # [H, NH]
    cz = np.einsum("nd,nd->n", np.asarray(bq, np.float64), kmask)  # [NH]
    z = q64.reshape(S * B, H) @ wz + cz  # [S*B, NH]
    choose = (z > 0).reshape(S, B, NH)

    # mask-branch output: mean of (unmasked-data) V over masked key positions
    v64 = np.asarray(value, np.float64)  # [S, B, H]
    vbar_feat = np.stack(
        [
            v64[mask[b], b, :].mean(axis=0)
            if mask[b].any()
            else np.zeros(H)
            for b in range(B)
        ]
    )  # [B, H]
    for b in range(B):
        if not mask[b].any():
            choose[:, b, :] = False  # no masked keys -> no mask branch
        elif mask[b].all():
            # all keys masked: identical scores -> uniform softmax -> Vbar
            choose[:, b, :] = True
    # bv is folded into bo on the device, so the host correction uses vbar
    # WITHOUT bv (the device adds bv@Wo to every row via the output bias).
    vbar = np.einsum("bh,nhd->bnd", vbar_feat, Wv64)  # [B, NH, DK]
    ubar = np.einsum(
        "bnd,ndh->bnh", vbar, Wo64.reshape(NH, DK, H)
    )  # [B, NH, H]

    # correction added on host for mask-branch rows
    ycorr = np.einsum("sbn,bnh->sbh", choose.astype(np.float64), ubar)

    # gather unmasked keys per batch
    idx = [np.nonzero(~mask[b])[0] for b in range(B)]
    umax = max(max(len(i) for i in idx), 1)
    UP = umax
    NKT = (UP + 127) // 128

    Wq_d = np.ascontiguousarray(
        np.asarray(Wq).transpose(1, 0, 2).reshape(H, NHDK)
    ).astype(npbf16)
    Wk_d = np.ascontiguousarray(
        np.asarray(Wk).transpose(1, 0, 2).reshape(H, NHDK)
    ).astype(npbf16)
    Wv_d = np.ascontiguousarray(
        np.asarray(Wv).transpose(1, 0, 2).reshape(H, NHDK)
    ).astype(npbf16)
    Wo_d = np.asarray(Wo, np.float32).astype(npbf16)
    bq_d = np.ascontiguousarray(np.asarray(bq, np.float32).T)  # [DK, NH]
    bk_d = np.ascontiguousarray(np.asarray(bk, np.float32).T)
    # fold bv into the output bias: y += bv_flat @ Wo (exact: softmax
    # weights sum to 1, so out = PV/d + bv before the output projection)
    bo_eff = np.asarray(bo, np.float64) + np.asarray(bv, np.float64).reshape(
        NHDK
    ) @ Wo64
    bo_d = np.ascontiguousarray(
        bo_eff.astype(np.float32).reshape(HT, 128).T
    )  # [128, HT]

    Sq = S // 2
    in_maps = []
    for core in range(NCORES):
        b, half = divmod(core, 2)
        qo = half * Sq
        ii = idx[b]
        u = len(ii)
        kuT = np.zeros((H, UP), npbf16)
        kuT[:, :u] = np.asarray(key[ii, b, :], np.float32).T.astype(npbf16)
        vuT = np.zeros((H, UP), npbf16)
        vuT[:, :u] = np.asarray(value[ii, b, :], np.float32).T.astype(npbf16)
        qT = np.ascontiguousarray(
            np.asarray(query[qo : qo + Sq, b, :], np.float32).T
        ).astype(npbf16)
        padb = np.zeros((128, NKT), np.float32)
        flat = np.arange(NKT * 128).reshape(NKT, 128).T  # [128, NKT] key index
        padb[flat >= max(u, 1)] = -30000.0  # keep >=1 live key (denom > 0)
        chb = np.ascontiguousarray(
            choose[qo : qo + Sq, b, :].T.astype(np.float32) * 1.0e30
        ).reshape(1, NH * Sq)
        in_maps.append(
            {
                "qT": qT,
                "kuT": kuT,
                "vuT": vuT,
                "wq": Wq_d,
                "wk": Wk_d,
                "wv": Wv_d,
                "wo": Wo_d,
                "bq": bq_d,
                "bk": bk_d,
                "bo": bo_d,
                "padb": padb,
                "chb": chb,
            }
        )
    return in_maps, ycorr, Sq, UP


def run(inputs: dict, trace: bool = False):
    in_maps, ycorr, Sq, UP = _prepare(**inputs)
    key_ = (Sq, UP)
    if key_ not in _PROG_CACHE:
        _PROG_CACHE[key_] = build_program(Sq, UP)
    nc = _PROG_CACHE[key_]
    res = run_bass_kernel_spmd(nc, in_maps, list(range(NCORES)), trace=trace)
    y = np.empty((S, B, H), np.float32)
    for core in range(NCORES):
        b, half = divmod(core, 2)
        qo = half * Sq
        y[qo : qo + Sq, b, :] = res.results[core]["yT"].T
    y += ycorr.astype(np.float32)
    return y, res


def kernel(**inputs) -> np.ndarray:
    y, _ = run(inputs, trace=False)
    return y


# revision 43
# speedup vs baseline: 1.0867x; 1.0482x over previous
"""Trainium2 Bass kernel for nn_MultiHeadHCGAttention.

Math notes (exact restructuring of the reference):
  The key_padding_mask replaces the ENTIRE key feature row with -1e9 BEFORE
  the K projection (v is NOT masked). Hence every masked key position s in
  batch b has the SAME projected K row:
      Kmask[n] = -1e9 * sum_h Wk[n,h,:] + bk[n]   (data independent)
  All masked keys share one score z = Q.Kmask/sqrt(dk) with |z| ~ 1e9.
  In fp32 softmax the output per (query q, head n) is therefore either
    - mean of V over the masked key positions  if z > max unmasked score
      (uniform softmax over the identical-score masked keys)
    - standard softmax over unmasked keys      otherwise (masked weights
      underflow to exactly 0 in fp32)
  The boundary band has probability ~1e-7 per query -> decided by sign(z),
  computed exactly on the host in fp64 (z = q @ (Wq@Kmask) + bq.Kmask).

  Device computes bf16 attention over the gathered unmasked keys only
  (normal O(1) magnitudes); rows whose head chose the mask branch get
  denom += 1e30 on device (output underflows to 0) and the contribution
  ubar[b,n] = (mean_masked V[b,n]) @ Wo_n is added on the host in fp64.
  bv is folded into bo on the host (softmax weights sum to 1 so
  out = PV/d + bv exactly before the output projection).

Sharding: 8 cores = (batch b in 0..3) x (query half). No collectives (the
pairwise AllGather fabric here moves ~38GB/s -- too slow to pay for
de-duplicating the K/V projections).

Schedule: projections and attention are interleaved so the Scalar engine's
exp stream (the attention-phase bottleneck) overlaps projection matmuls,
and attention runs qc-major with the first query-chunk's output projection
emitted piecewise between second-chunk heads. The softmax denominator is a
DVE bf16 chain over the exp tiles + one 512-cycle matmul per chunk
(deferred one chunk so the PE never waits on the chain).
"""

import math
import sys

if "/opt/trn_rl_repo" not in sys.path:
    sys.path.insert(0, "/opt/trn_rl_repo")

import ml_dtypes
import numpy as np

import concourse.bacc as bacc
import concourse.tile as tile
from concourse import mybir
from concourse.bass_utils import run_bass_kernel_spmd

S, B, H = 2048, 4, 1024
NH, DK = 8, 128
NHDK = NH * DK
NEG = -1.0e9
NCORES = 8
HT = H // 128  # 8 H-tiles

bf16 = mybir.dt.bfloat16
f32 = mybir.dt.float32
npbf16 = ml_dtypes.bfloat16

_PROG_CACHE: dict = {}


def build_program(Sq: int, UP: int):
    """Emit the per-core SPMD program. Sq = queries per core, UP =
    unmasked-key count (max over batches)."""
    NKT = (UP + 127) // 128
    ktiles = [(o, min(128, UP - o)) for o in range(0, UP, 128)]
    QC = Sq // 512  # 512-wide query chunks
    # key free-dim chunks for the K projection
    kchunks = []
    o = 0
    while o < UP:
        w = min(512, UP - o)
        kchunks.append((o, w))
        o += w

    nc = bacc.Bacc("TRN2", target_bir_lowering=False, debug=False)

    d_qT = nc.dram_tensor("qT", [H, Sq], bf16, kind="ExternalInput")
    d_kuT = nc.dram_tensor("kuT", [H, UP], bf16, kind="ExternalInput")
    d_vuT = nc.dram_tensor("vuT", [H, UP], bf16, kind="ExternalInput")
    d_wq = nc.dram_tensor("wq", [H, NHDK], bf16, kind="ExternalInput")
    d_wk = nc.dram_tensor("wk", [NH * 128, HT * 128], bf16, kind="ExternalInput")
    d_wv = nc.dram_tensor("wv", [H, NHDK], bf16, kind="ExternalInput")
    d_wo = nc.dram_tensor("wo", [NHDK, H], bf16, kind="ExternalInput")
    d_bq = nc.dram_tensor("bq", [DK, NH], f32, kind="ExternalInput")
    d_bk = nc.dram_tensor("bk", [DK, NH], f32, kind="ExternalInput")
    d_bo = nc.dram_tensor("bo", [128, HT], f32, kind="ExternalInput")
    d_padb = nc.dram_tensor("padb", [128, NKT], f32, kind="ExternalInput")
    d_chb = nc.dram_tensor("chb", [1, NH * Sq], bf16, kind="ExternalInput")
    d_yT = nc.dram_tensor("yT", [H, Sq], f32, kind="ExternalOutput")

    SCALE = 1.0 / math.sqrt(DK)

    with tile.TileContext(nc) as tc:
        with (
            tc.tile_pool(name="const", bufs=1) as const,
            tc.tile_pool(name="exp", bufs=3) as expp,
            tc.tile_pool(name="es", bufs=3) as esp,
            tc.tile_pool(name="sc", bufs=2) as scp,
            tc.tile_pool(name="bc", bufs=3) as bcp,
            tc.tile_pool(name="yt", bufs=2) as ytp,
            tc.tile_pool(name="ps_proj", bufs=4, space="PSUM") as ps_proj,
            tc.tile_pool(name="ps_pv", bufs=3, space="PSUM") as ps_pv,
            tc.tile_pool(name="ps_d", bufs=1, space="PSUM") as ps_d,
        ):
            qT = const.tile([128, HT, Sq], bf16)
            kuT = const.tile([128, HT, UP], bf16)
            vuT = const.tile([128, HT, UP], bf16)
            wq = const.tile([128, HT, NHDK], bf16)
            wk = const.tile([128, NH, HT, 128], bf16)
            wv = const.tile([128, HT, NHDK], bf16)
            wo = const.tile([128, NH, H], bf16)
            bq = const.tile([128, NH], f32)
            bk = const.tile([128, NH], f32)
            bo = const.tile([128, HT], f32)
            padb = const.tile([128, NKT], f32)
            ones_mat = const.tile([128, 128], bf16)
            nc.vector.memset(ones_mat[:], 1.0)
            ksb = const.tile([128, NH, UP], bf16)
            vg = const.tile([128, 2, NKT, 512], bf16)
            qsb = const.tile([128, NH, Sq], bf16)
            out_all = const.tile([128, NH, Sq], bf16)

            r_qT = d_qT[:].rearrange("(t p) s -> p t s", p=128)
            r_kuT = d_kuT[:].rearrange("(t p) u -> p t u", p=128)
            r_vuT = d_vuT[:].rearrange("(t p) u -> p t u", p=128)
            r_wq = d_wq[:].rearrange("(t p) d -> p t d", p=128)
            r_wk = d_wk[:].rearrange("(n p) (t c) -> p n t c", p=128, t=HT)
            r_wv = d_wv[:].rearrange("(t p) d -> p t d", p=128)
            r_wo = d_wo[:].rearrange("(n p) h -> p n h", p=128)

            # Single SP HWDGE ring, strict consumption order (FIFO, split
            # across all 16 SDMA engines). kuT/wk interleaved per H-tile so
            # the first kproj chain starts ASAP; later groups batched into
            # half-tensor transfers to cut trigger overhead on the queue.
            for ht in range(HT):
                nc.sync.dma_start(kuT[:, ht, :], r_kuT[:, ht, :])
            # wk is laid out head-major in DRAM (host transposes) so each
            # head's weights transfer as one full-burst contiguous DMA:
            # kproj(0) waits only on kuT + 0.26MB instead of the full 2.1MB
            for n in range(NH):
                nc.sync.dma_start(wk[:, n, :, :], r_wk[:, n, :, :])
                if n == 0:
                    nc.sync.dma_start(bk[:], d_bk[:])
            nc.sync.dma_start(bq[:], d_bq[:])
            nc.sync.dma_start(bo[:], d_bo[:])
            nc.sync.dma_start(padb[:], d_padb[:])
            for h0 in (0, 4):
                nc.sync.dma_start(vuT[:, h0 : h0 + 4, :], r_vuT[:, h0 : h0 + 4, :])
                nc.sync.dma_start(wv[:, h0 : h0 + 4, :], r_wv[:, h0 : h0 + 4, :])
            for h0 in (0, 4):
                nc.sync.dma_start(qT[:, h0 : h0 + 4, :], r_qT[:, h0 : h0 + 4, :])
                nc.sync.dma_start(wq[:, h0 : h0 + 4, :], r_wq[:, h0 : h0 + 4, :])
            for n0 in (0, 4):
                nc.sync.dma_start(wo[:, n0 : n0 + 4, :], r_wo[:, n0 : n0 + 4, :])

            def kproj(n):
                for o, w in kchunks:
                    pk = ps_proj.tile([128, 512], f32, tag="proj")
                    for ht in range(HT):
                        nc.tensor.matmul(
                            pk[:, :w],
                            wk[:, n, ht, :],
                            kuT[:, ht, o : o + w],
                            start=(ht == 0),
                            stop=(ht == HT - 1),
                        )
                    nc.vector.tensor_scalar_add(
                        ksb[:, n, o : o + w], pk[:, :w], bk[:, n : n + 1]
                    )

            def vproj_tile(g, kt):
                o, klen = ktiles[kt]
                pv = ps_proj.tile([128, 512], f32, tag="proj")
                for ht in range(HT):
                    nc.tensor.matmul(
                        pv[:klen],
                        vuT[:, ht, o : o + klen],
                        wv[:, ht, g * 512 : (g + 1) * 512],
                        start=(ht == 0),
                        stop=(ht == HT - 1),
                    )
                nc.vector.tensor_copy(vg[:klen, g, kt, :], pv[:klen])

            def vproj(g):
                for kt in range(NKT):
                    vproj_tile(g, kt)

            def qproj(n):
                for qc in range(QC):
                    pq = ps_proj.tile([128, 512], f32, tag="proj")
                    for ht in range(HT):
                        nc.tensor.matmul(
                            pq[:],
                            wq[:, ht, n * 128 : (n + 1) * 128],
                            qT[:, ht, qc * 512 : (qc + 1) * 512],
                            start=(ht == 0),
                            stop=(ht == HT - 1),
                        )
                    nc.vector.tensor_scalar_add(
                        qsb[:, n, qc * 512 : (qc + 1) * 512], pq[:], bq[:, n : n + 1]
                    )

            def attn_chunk(n, qc):
                """Scores + exp + PV for one (head, 512-query) chunk. The
                softmax denominator accumulates as a DVE bf16 chain over the
                exp tiles; its single matmul is deferred one chunk so the PE
                never waits on the chain."""
                qsl = slice(qc * 512, (qc + 1) * 512)
                chb = bcp.tile([128, 512], bf16, tag="chb")
                nc.gpsimd.dma_start(
                    chb[:],
                    d_chb[
                        0:1, n * Sq + qc * 512 : n * Sq + (qc + 1) * 512
                    ].to_broadcast([128, 512]),
                )
                ppv = ps_pv.tile([128, 512], f32)
                esum = esp.tile([128, 512], bf16)
                e0 = None
                k0 = 0
                for kt, (ko, klen) in enumerate(ktiles):
                    ps = ps_proj.tile([128, 512], f32, tag="proj")
                    nc.tensor.matmul(
                        ps[:klen],
                        ksb[:, n, ko : ko + klen],
                        qsb[:, n, qsl],
                        start=True,
                        stop=True,
                    )
                    e = expp.tile([128, 512], bf16)
                    nc.scalar.activation(
                        out=e[:klen],
                        in_=ps[:klen],
                        func=mybir.ActivationFunctionType.Exp,
                        bias=padb[:klen, kt : kt + 1],
                        scale=SCALE,
                    )
                    nc.tensor.matmul(
                        ppv[:],
                        vg[:klen, n // 4, kt, n % 4 * 128 : (n % 4 + 1) * 128],
                        e[:klen],
                        start=(kt == 0),
                        stop=(kt == NKT - 1),
                    )
                    if kt == 0:
                        e0, k0 = e, klen
                    elif kt == 1:
                        nc.vector.tensor_add(esum[:klen], e0[:klen], e[:klen])
                        if klen < k0:
                            nc.vector.tensor_copy(esum[klen:k0], e0[klen:k0])
                    else:
                        nc.vector.tensor_add(esum[:klen], esum[:klen], e[:klen])
                if NKT == 1:
                    nc.vector.tensor_copy(esum[:k0], e0[:k0])
                return (n, qsl, ppv, esum, chb)

            KMAX = min(128, UP)

            def attn_finish(st):
                n, qsl, ppv, esum, chb = st
                pd = ps_d.tile([128, 512], f32)
                nc.tensor.matmul(
                    pd[:], ones_mat[:KMAX], esum[:KMAX], start=True, stop=True
                )
                # mask-branch rows get denom += 1e30: output underflows to 0
                pda = scp.tile([128, 512], f32, tag="pda")
                nc.vector.tensor_add(pda[:], pd[:], chb[:])
                rec = scp.tile([128, 512], f32, tag="rec")
                nc.vector.reciprocal_approx_fast(rec[:], pda[:])
                nc.vector.tensor_mul(out_all[:, n, qsl], ppv[:], rec[:])

            r_yT = d_yT[:].rearrange("(t p) s -> t p s", p=128)

            def outproj_piece(qc, ht):
                py = ps_proj.tile([128, 512], f32, tag="proj")
                for n in range(NH):
                    nc.tensor.matmul(
                        py[:],
                        wo[:, n, ht * 128 : (ht + 1) * 128],
                        out_all[:, n, qc * 512 : (qc + 1) * 512],
                        start=(n == 0),
                        stop=(n == NH - 1),
                    )
                yt = ytp.tile([128, 512], f32)
                nc.vector.tensor_scalar_add(yt[:], py[:], bo[:, ht : ht + 1])
                nc.sync.dma_start(
                    r_yT[ht, :, qc * 512 : (qc + 1) * 512], yt[:]
                )

            # ---- interleaved schedule ----
            # Projections and attention run in coarse blocks: fine-grained
            # interleaving was measured ~15% SLOWER per-op across all
            # engines (SBUF/PSUM port contention). Batch 1 of projections,
            # then attention chunks (qc0, heads 0-3) interleaved with batch
            # 2, then (qc0, heads 4-7), then qc1 with qc0's output
            # projection pieces spliced between heads. attn_finish trails
            # two chunks behind so the PE never waits on the DVE chain.
            pending = []

            def emit_chunk(n, qc):
                st = attn_chunk(n, qc)
                pending.append(st)
                if len(pending) > 2:
                    attn_finish(pending.pop(0))

            for n in range(4):
                kproj(n)
            vproj(0)
            for n in range(4):
                qproj(n)
            for j, n in enumerate(range(4)):
                kproj(4 + j)
                emit_chunk(n, 0)
            vproj(1)
            for j, n in enumerate(range(4, NH)):
                qproj(4 + j)
                emit_chunk(n, 0)
            for n in range(NH):
                emit_chunk(n, 1)
                if n >= 1:
                    outproj_piece(0, n - 1)
            while pending:
                attn_finish(pending.pop(0))
            outproj_piece(0, HT - 1)
            for ht in range(HT):
                outproj_piece(1, ht)

    nc.compile()
    return nc


def _prepare(query, key, value, key_padding_mask, Wq, bq, Wk, bk, Wv, bv, Wo, bo):
    """Host-side prep: mask constants (fp64), gather/transpose, per-core maps."""
    mask = np.asarray(key_padding_mask)
    q64 = np.asarray(query, np.float64)
    Wq64 = np.asarray(Wq, np.float64)
    Wk64 = np.asarray(Wk, np.float64)
    Wv64 = np.asarray(Wv, np.float64)
    Wo64 = np.asarray(Wo, np.float64)

    # shared projected row of all masked keys, per head
    kmask = NEG * Wk64.sum(axis=1) + np.asarray(bk, np.float64)  # [NH, DK]

    # z sign per (s, b, n):  z = q . (Wq[n] @ kmask[n]) + bq[n].kmask[n]
    wz = np.einsum("nhd,nd->hn", Wq64, kmask)  # [H, NH]
    cz = np.einsum("nd,nd->n", np.asarray(bq, np.float64), kmask)  # [NH]
    z = q64.reshape(S * B, H) @ wz + cz  # [S*B, NH]
    choose = (z > 0).reshape(S, B, NH)

    # mask-branch output: mean of (unmasked-data) V over masked key positions
    v64 = np.asarray(value, np.float64)  # [S, B, H]
    vbar_feat = np.stack(
        [
            v64[mask[b], b, :].mean(axis=0)
            if mask[b].any()
            else np.zeros(H)
            for b in range(B)
        ]
    )  # [B, H]
    for b in range(B):
        if not mask[b].any():
            choose[:, b, :] = False  # no masked keys -> no mask branch
        elif mask[b].all():
            # all keys masked: identical scores -> uniform softmax -> Vbar
            choose[:, b, :] = True
    # bv is folded into bo on the device, so the host correction uses vbar
    # WITHOUT bv (the device adds bv@Wo to every row via the output bias).
    vbar = np.einsum("bh,nhd->bnd", vbar_feat, Wv64)  # [B, NH, DK]
    ubar = np.einsum(
        "bnd,ndh->bnh", vbar, Wo64.reshape(NH, DK, H)
    )  # [B, NH, H]

    # correction added on host for mask-branch rows
    ycorr = np.einsum("sbn,bnh->sbh", choose.astype(np.float64), ubar)

    # gather unmasked keys per batch
    idx = [np.nonzero(~mask[b])[0] for b in range(B)]
    umax = max(max(len(i) for i in idx), 1)
    UP = umax
    NKT = (UP + 127) // 128

    Wq_d = np.ascontiguousarray(
        np.asarray(Wq).transpose(1, 0, 2).reshape(H, NHDK)
    ).astype(npbf16)
    # head-major, partition-major layout: Wk_d[n, p, ht, c] = Wk[n, ht*128+p, c]
    Wk_d = np.ascontiguousarray(
        np.asarray(Wk).reshape(NH, HT, 128, DK).transpose(0, 2, 1, 3)
    ).reshape(NH * 128, HT * DK).astype(npbf16)
    Wv_d = np.ascontiguousarray(
        np.asarray(Wv).transpose(1, 0, 2).reshape(H, NHDK)
    ).astype(npbf16)
    Wo_d = np.asarray(Wo, np.float32).astype(npbf16)
    bq_d = np.ascontiguousarray(np.asarray(bq, np.float32).T)  # [DK, NH]
    bk_d = np.ascontiguousarray(np.asarray(bk, np.float32).T)
    # fold bv into the output bias: y += bv_flat @ Wo (exact: softmax
    # weights sum to 1, so out = PV/d + bv before the output projection)
    bo_eff = np.asarray(bo, np.float64) + np.asarray(bv, np.float64).reshape(
        NHDK
    ) @ Wo64
    bo_d = np.ascontiguousarray(
        bo_eff.astype(np.float32).reshape(HT, 128).T
    )  # [128, HT]

    Sq = S // 2
    in_maps = []
    for core in range(NCORES):
        b, half = divmod(core, 2)
        qo = half * Sq
        ii = idx[b]
        u = len(ii)
        kuT = np.zeros((H, UP), npbf16)
        kuT[:, :u] = np.asarray(key[ii, b, :], np.float32).T.astype(npbf16)
        vuT = np.zeros((H, UP), npbf16)
        vuT[:, :u] = np.asarray(value[ii, b, :], np.float32).T.astype(npbf16)
        qT = np.ascontiguousarray(
            np.asarray(query[qo : qo + Sq, b, :], np.float32).T
        ).astype(npbf16)
        padb = np.zeros((128, NKT), np.float32)
        flat = np.arange(NKT * 128).reshape(NKT, 128).T  # [128, NKT] key index
        padb[flat >= max(u, 1)] = -30000.0  # keep >=1 live key (denom > 0)
        chb = np.ascontiguousarray(
            (choose[qo : qo + Sq, b, :].T.astype(np.float32) * 1.0e30).astype(
                npbf16
            )
        ).reshape(1, NH * Sq)
        in_maps.append(
            {
                "qT": qT,
                "kuT": kuT,
                "vuT": vuT,
                "wq": Wq_d,
                "wk": Wk_d,
                "wv": Wv_d,
                "wo": Wo_d,
                "bq": bq_d,
                "bk": bk_d,
                "bo": bo_d,
                "padb": padb,
                "chb": chb,
            }
        )
    return in_maps, ycorr, Sq, UP


def run(inputs: dict, trace: bool = False):
    in_maps, ycorr, Sq, UP = _prepare(**inputs)
    key_ = (Sq, UP)
    if key_ not in _PROG_CACHE:
        _PROG_CACHE[key_] = build_program(Sq, UP)
    nc = _PROG_CACHE[key_]
    res = run_bass_kernel_spmd(nc, in_maps, list(range(NCORES)), trace=trace)
    y = np.empty((S, B, H), np.float32)
    for core in range(NCORES):
        b, half = divmod(core, 2)
        qo = half * Sq
        y[qo : qo + Sq, b, :] = res.results[core]["yT"].T
    y += ycorr.astype(np.float32)
    return y, res


def kernel(**inputs) -> np.ndarray:
    y, _ = run(inputs, trace=False)
    return y
